# revision 1
# baseline (speedup 1.0000x reference)
"""Trainium2 Bass kernel: causal multi-head attention with RoPE.

Reference computation (B=2, T=2048, C=2048, H=16, D=128, fp32):
    q/k/v = hs @ {q,k,v}_w^T ; RoPE(q), RoPE(k)
    out   = softmax(causal(q k^T / sqrt(D))) v @ o_w^T

Sharding: tensor-parallel over heads — each of the 8 cores owns 2 heads.
Each core computes its heads' projections + attention and a partial output
projection; the host sums the 8 partials (bf16 partials, fp32 sum).

Per-core device pipeline (all matmuls in float32r = full-rate fp32):
  A) stream hs^T chunks; qT/kT in [d, t] layout (per-window tiles); v in
     [t, d] layout; RoPE (rotate_half as a +-1 permutation matmul +
     cos/sin elementwise).  The very first window/batch uses per-chunk
     tiles and DMAs spread over 4 engine queues so the first matmul
     starts ~2.5us in instead of ~20us.
  B) scores TRANSPOSED [tk, tq] with causal N-trimming: diagonal k-tiles
     only compute columns tq >= 128*off (min N=256 to keep fp32r at
     1 cyc/row); exp on ACT; masking only the boundary 128-wide block.
     Softmax denominator WITHOUT per-tile ones-matmuls: e-tiles are
     pair-summed into two accumulators (bulk tiles on DVE, the 4
     diagonal tiles on GPSIMD), then TWO chained ones-matmuls per
     window produce the den row; DVE IEEE reciprocal on the row; gpsimd
     partition-broadcast; the normalize multiply is FUSED into the
     PSUM->SBUF attnT copy.  Windows processed [1,2,3,0] so the final
     (tail-exposed) window is the cheapest one.
  C) output projection interleaved between attention windows; partial
     [t, c] tiles cast to bf16 and DMA'd out (halves the 33.5MB/core
     output traffic; host sums in fp32).
"""

import math
import sys

if "/opt/trn_rl_repo" not in sys.path:
    sys.path.insert(0, "/opt/trn_rl_repo")

import numpy as np

import concourse.bass as bass
import concourse.mybir as mybir
import concourse.tile as tile
from concourse import bacc, bass_utils

F32 = mybir.dt.float32
F32R = mybir.dt.float32r
BF16 = mybir.dt.bfloat16
AF = mybir.ActivationFunctionType
MULT = mybir.AluOpType.mult
ADD = mybir.AluOpType.add

B = 2
C = 2048
H = 16
D = 128
N_CORES = 8
HPC = H // N_CORES  # heads per core
DPC = HPC * D  # channels per core (256)
ROPE_BASE = 10000.0
P = 128  # partitions
TQW = 512  # tq window (matmul free dim)
TCH = 256  # hs^T chunk width in t


def _build_nc(T: int = 2048):
    """Build the per-core Bass program (SPMD: same program, per-core data)."""
    KT = C // P  # 16 k-tiles over the contraction dim c
    n_w = T // TQW  # tq windows per (b, h)
    spw = TQW // P  # 128-row subtiles per window (4)
    scale = 1.0 / math.sqrt(D)

    nc = bacc.Bacc(trn_type="TRN2", target_bir_lowering=False, debug=False)

    hst = nc.dram_tensor("hst", [B, P, T // TQW, KT // 4, 4, TQW], BF16, kind="ExternalInput").ap()
    wq = nc.dram_tensor("wq_t", [P, KT, DPC], BF16, kind="ExternalInput").ap()
    wk = nc.dram_tensor("wk_t", [P, KT, DPC], BF16, kind="ExternalInput").ap()
    wv = nc.dram_tensor("wv_t", [P, KT, DPC], BF16, kind="ExternalInput").ap()
    ow = nc.dram_tensor("ow_t", [P, HPC, C], F32R, kind="ExternalInput").ap()
    cos_d = nc.dram_tensor("cos_t", [D, T], F32, kind="ExternalInput").ap()
    sin_d = nc.dram_tensor("sin_t", [D, T], F32, kind="ExternalInput").ap()
    rp_d = nc.dram_tensor("rperm", [D, D], F32R, kind="ExternalInput").ap()
    ones_d = nc.dram_tensor("ones", [P, 1], F32R, kind="ExternalInput").ap()
    msk_d = nc.dram_tensor("masks", [P, 384], F32, kind="ExternalInput").ap()
    out_d = nc.dram_tensor("out_p", [B, T // P, C // TQW, P, TQW], BF16, kind="ExternalOutput").ap()

    with tile.TileContext(nc) as tc:
        with (
            tc.tile_pool(name="consts", bufs=1) as consts,
            tc.tile_pool(name="hst", bufs=4) as hstp,
            tc.tile_pool(name="qkv", bufs=1) as qkvp,
            tc.tile_pool(name="exp", bufs=6) as expp,
            tc.tile_pool(name="esum", bufs=2) as esump,
            tc.tile_pool(name="bc", bufs=3) as bcp,
            tc.tile_pool(name="small", bufs=2) as smallp,
            tc.tile_pool(name="outp", bufs=8) as outp,
            tc.tile_pool(name="psA", bufs=4, space="PSUM") as psA,
            tc.tile_pool(name="psB", bufs=4, space="PSUM") as psB,
        ):
            # ---- resident constants -------------------------------------
            # First 4 contraction chunks of each weight are separate tiles
            # (fine-grained arrival for the kernel head); the remaining 12
            # keep quarter granularity.
            wq_c = [consts.tile([P, DPC], BF16, tag=f"wqc{k}", name=f"wqc{k}") for k in range(4)]
            wk_c = [consts.tile([P, DPC], BF16, tag=f"wkc{k}", name=f"wkc{k}") for k in range(4)]
            wv_c = [consts.tile([P, DPC], BF16, tag=f"wvc{k}", name=f"wvc{k}") for k in range(4)]
            wq_q = [consts.tile([P, 4, DPC], BF16, tag=f"wqq{i}", name=f"wqq{i}") for i in range(1, 4)]
            wk_q = [consts.tile([P, 4, DPC], BF16, tag=f"wkq{i}", name=f"wkq{i}") for i in range(1, 4)]
            wv_q = [consts.tile([P, 4, DPC], BF16, tag=f"wvq{i}", name=f"wvq{i}") for i in range(1, 4)]
            ow_sb = consts.tile([P, HPC, C], F32R, tag="ow")
            cos_sb = consts.tile([D, T], F32, tag="cos")
            sin_sb = consts.tile([D, T], F32, tag="sin")
            msk_sb = consts.tile([P, 384], F32, tag="msk")
            ones_sb = consts.tile([P, 1], F32R, tag="ones")
            rp_sb = consts.tile([D, D], F32R, tag="rp")

            def wslc(w_c, w_q, k, h=None):
                if k < 4:
                    t = w_c[k]
                    return t[:, bass.ts(h, D)] if h is not None else t[:]
                t = w_q[k // 4 - 1]
                return (
                    t[:, k % 4, bass.ts(h, D)] if h is not None else t[:, k % 4, :]
                )

            # Critical-path-first DMA order, spread over 4 issue queues so
            # the first matmuls (k=0..3 of window 0) can start early.  The
            # first window's hs^T quarters are DMA'd per chunk (4 smaller
            # transfers into slices of each quarter tile).
            pre_tiles = [
                hstp.tile([P, 4, TQW], BF16, tag="hst", name="ht_pre")
                for _ in range(4)
            ]
            nc.scalar.dma_start(wq_c[0][:], wq[:, 0, :])
            nc.gpsimd.dma_start(wk_c[0][:], wk[:, 0, :])
            nc.sync.dma_start(pre_tiles[0][:, 0, :], hst[0, :, 0, 0, 0, :])
            nc.scalar.dma_start(wq_c[1][:], wq[:, 1, :])
            nc.gpsimd.dma_start(wk_c[1][:], wk[:, 1, :])
            nc.sync.dma_start(pre_tiles[0][:, 1, :], hst[0, :, 0, 0, 1, :])
            nc.scalar.dma_start(wq_c[2][:], wq[:, 2, :])
            nc.scalar.dma_start(wk_c[2][:], wk[:, 2, :])
            nc.sync.dma_start(pre_tiles[0][:, 2, :], hst[0, :, 0, 0, 2, :])
            nc.scalar.dma_start(wq_c[3][:], wq[:, 3, :])
            nc.scalar.dma_start(wk_c[3][:], wk[:, 3, :])
            nc.sync.dma_start(pre_tiles[0][:, 3, :], hst[0, :, 0, 0, 3, :])
            nc.gpsimd.dma_start(wk_q[0][:], wk[:, bass.ts(1, 4), :])
            nc.scalar.dma_start(wq_q[0][:], wq[:, bass.ts(1, 4), :])
            for k in range(4, 8):
                nc.sync.dma_start(pre_tiles[1][:, k % 4, :], hst[0, :, 0, 1, k % 4, :])
            nc.gpsimd.dma_start(wk_q[1][:], wk[:, bass.ts(2, 4), :])
            nc.scalar.dma_start(wq_q[1][:], wq[:, bass.ts(2, 4), :])
            nc.sync.dma_start(pre_tiles[2][:], hst[0, :, 0, 2, :, :])
            nc.gpsimd.dma_start(wk_q[2][:], wk[:, bass.ts(3, 4), :])
            nc.scalar.dma_start(wq_q[2][:], wq[:, bass.ts(3, 4), :])
            nc.sync.dma_start(pre_tiles[3][:], hst[0, :, 0, 3, :, :])
            nc.scalar.dma_start(rp_sb[:], rp_d)
            for k in range(4):
                nc.sync.dma_start(wv_c[k][:], wv[:, k, :])
            for i in range(3):
                nc.sync.dma_start(wv_q[i][:], wv[:, bass.ts(i + 1, 4), :])
            nc.scalar.dma_start(cos_sb[:], cos_d)
            nc.scalar.dma_start(sin_sb[:], sin_d)
            nc.scalar.dma_start(msk_sb[:], msk_d)
            nc.scalar.dma_start(ones_sb[:], ones_d)
            late_dmas_done = []

            for b in range(B):
                # Per-window q/k tiles: fine-grained deps (a window's
                # consumers only wait on that window's producers).
                q_t = [
                    [qkvp.tile([P, TQW], F32R, tag=f"q{h}w{w}", name=f"q{h}w{w}") for w in range(n_w)]
                    for h in range(HPC)
                ]
                k_t = [
                    [qkvp.tile([P, TQW], F32R, tag=f"k{h}w{w}", name=f"k{h}w{w}") for w in range(n_w)]
                    for h in range(HPC)
                ]
                v_sb = qkvp.tile([P, T // P, DPC], F32R, tag="v")

                # ---- phase A: projections + RoPE ------------------------
                def rope(w, b=b):
                    sl = bass.ts(w, TQW)
                    for h in range(HPC):
                        for x_t in (q_t, k_t):
                            x = x_t[h][w]
                            rh = psB.tile([P, TQW], F32, tag="psB", name="rh")
                            nc.tensor.matmul(
                                rh[:], rp_sb[:], x[:], start=True, stop=True
                            )
                            t1 = smallp.tile([P, TQW], F32, tag="t1")
                            nc.vector.tensor_tensor(
                                t1[:], x[:].bitcast(F32), cos_sb[:, sl], op=MULT
                            )
                            nc.vector.tensor_tensor(rh[:], rh[:], sin_sb[:, sl], op=MULT)
                            nc.vector.tensor_tensor(x[:], t1[:], rh[:], op=ADD)

                ctx_a = nc.named_scope(f"A{b}"); ctx_a.__enter__()
                for w in range(n_w):
                    if b == 0 and w == 0:
                        hts = pre_tiles
                    else:
                        hts = []
                        for qi in range(4):
                            ht = hstp.tile([P, 4, TQW], BF16, tag="hst", name="ht")
                            nc.sync.dma_start(ht[:], hst[b, :, w, qi, :, :])
                            hts.append(ht)
                    hsl = [hts[k // 4][:, k % 4, :] for k in range(KT)]
                    hsl_sub = lambda k, sub, hts=hts: hts[k // 4][:, k % 4, bass.ts(sub, P)]
                    pq = [psA.tile([P, TQW], F32, tag="psA", name="pq") for _ in range(HPC)]
                    pk = [psA.tile([P, TQW], F32, tag="psA", name="pk") for _ in range(HPC)]
                    for k in range(KT):
                        for h in range(HPC):
                            for pt, w_cq in ((pq[h], (wq_c, wq_q)), (pk[h], (wk_c, wk_q))):
                                nc.tensor.matmul(
                                    pt[:],
                                    wslc(w_cq[0], w_cq[1], k, h),
                                    hsl[k],
                                    start=(k == 0),
                                    stop=(k == KT - 1),
                                )
                    # Rank the psum->sbuf copies later so attention's first
                    # exps win the ACT queue at the phase A->B transition
                    # (deps still force early-window copies on time).
                    with tc.high_priority(-2000):
                        for h in range(HPC):
                            nc.scalar.activation(q_t[h][w][:], pq[h][:], AF.Copy)
                            nc.scalar.activation(k_t[h][w][:], pk[h][:], AF.Copy)
                    pv4 = [
                        psB.tile([P, DPC], F32, tag="psB", name="pv4")
                        for _ in range(spw)
                    ]
                    for k in range(KT):
                        for sub in range(spw):
                            nc.tensor.matmul(
                                pv4[sub][:],
                                hsl_sub(k, sub),
                                wslc(wv_c, wv_q, k),
                                start=(k == 0),
                                stop=(k == KT - 1),
                            )
                    with tc.high_priority(-2000):
                        for sub in range(spw):
                            nc.scalar.activation(
                                v_sb[:, w * spw + sub, :], pv4[sub][:], AF.Copy
                            )
                    rope(w)
                ctx_a.__exit__(None, None, None)

                # ---- phase B: attention -------------------------------
                # Diagonal k-tile column trim: tile with offset `off`
                # (0..3) only needs columns tq >= 128*off; fp32r needs
                # N >= 256, so off=3 computes [256:512] and relies on the
                # wide mask to zero [256:384] before PV/den.
                def attend_win(h, w):
                    ntk = (w + 1) * spw

                    def qk_exp(i, h=h, w=w):
                        off = i - w * spw
                        c0 = 0 if off <= 0 else (256 if off == 3 else P * off)
                        st = psB.tile([P, TQW], F32, tag="psB")
                        nc.tensor.matmul(
                            st[:, c0:],
                            k_t[h][i // spw][:, bass.ts(i % spw, P)],
                            q_t[h][w][:, c0:],
                            start=True,
                            stop=True,
                        )
                        e = expp.tile([P, TQW], F32R, tag="exp")
                        nc.scalar.activation(e[:, c0:], st[:, c0:], AF.Exp, scale=scale)
                        if off >= 0:
                            if off == 3:
                                nc.vector.tensor_tensor(
                                    e[:, 256:], e[:, 256:].bitcast(F32),
                                    msk_sb[:, 128:384], op=MULT,
                                )
                            else:
                                o0 = P * off
                                nc.vector.tensor_tensor(
                                    e[:, o0:o0 + P], e[:, o0:o0 + P].bitcast(F32),
                                    msk_sb[:, 0:P], op=MULT,
                                )
                        return e, c0
                    # NOTE: mask outputs write the F32R AP directly (no
                    # bitcast) so the DVE rounds to fp32r for the PV matmul.

                    fifo = [qk_exp(j) for j in range(min(3, ntk))]
                    pv = psA.tile([P, TQW], F32, tag="psA")
                    den = psA.tile([P, TQW], F32, tag="psA")
                    nfull = ntk - spw  # leading full-width tiles
                    # Denominator: DVE pair-sums of full-width e-tiles feed
                    # one ones-matmul per pair; the 4 diagonal tiles get
                    # individual N-trimmed ones-matmuls.  Jobs are emitted
                    # one behind the PV stream so nothing stalls.  Natural
                    # order puts a full-width operand first (pair0, or
                    # w=0's off-0 tile), as required for start=True.
                    pair_pend = None
                    jobs = []  # (ap, c0) pending den matmuls
                    n_jobs_total = nfull // 2 + spw
                    emitted = [0]

                    def den_mm():
                        ap, c0 = jobs.pop(0)
                        nc.tensor.matmul(
                            den[:1, c0:],
                            ones_sb[:],
                            ap,
                            start=(emitted[0] == 0),
                            stop=(emitted[0] == n_jobs_total - 1),
                        )
                        emitted[0] += 1

                    for i in range(ntk):
                        if i + 3 < ntk:
                            fifo.append(qk_exp(i + 3))
                        e, c0 = fifo.pop(0)
                        nc.tensor.matmul(
                            pv[:, c0:],
                            v_sb[:, i, bass.ts(h, D)],
                            e[:, c0:],
                            start=(i == 0),
                            stop=(i == ntk - 1),
                        )
                        if i < nfull:
                            if pair_pend is None:
                                pair_pend = e
                            else:
                                pr = esump.tile(
                                    [P, TQW], F32R, tag="pair", name="pair"
                                )
                                nc.vector.tensor_tensor(
                                    pr[:],
                                    pair_pend[:].bitcast(F32),
                                    e[:].bitcast(F32),
                                    op=ADD,
                                )
                                pair_pend = None
                                jobs.append((pr[:], 0))
                        else:
                            jobs.append((e[:, c0:], c0))
                        # drain den jobs, keeping two in flight for slack
                        while len(jobs) > 2:
                            den_mm()
                    while jobs:
                        den_mm()

                    # reciprocal immediately (frees the den PSUM slot);
                    # broadcast + normalize are deferred one attend-step by
                    # the caller to avoid engine convoys
                    bc = bcp.tile([P, TQW], F32, tag="bc", name="bc")
                    nc.vector.reciprocal(bc[:1, :], den[:1, :])

                    def finalize(h=h, w=w, pv=pv, bc=bc):
                        nc.gpsimd.partition_broadcast(bc[:], bc[:1, :])
                        nc.vector.tensor_tensor(
                            q_t[h][w][:], pv[:], bc[:], op=MULT
                        )
                    return finalize

                def phase_c_win(w, half=None):
                    ms = range(w * spw, (w + 1) * spw)
                    if half is not None:
                        ms = ms[: len(ms) // 2] if half == 0 else ms[len(ms) // 2 :]
                    for m in ms:
                        for n in range(C // TQW):
                            pool = psA if n % 2 == 0 else psB
                            po = pool.tile([P, TQW], F32, tag=pool.name, name="po")
                            for h in range(HPC):
                                nc.tensor.matmul(
                                    po[:],
                                    q_t[h][m // spw][:, bass.ts(m % spw, P)],
                                    ow_sb[:, h, bass.ts(n, TQW)],
                                    start=(h == 0),
                                    stop=(h == HPC - 1),
                                )
                            o_t = outp.tile([P, TQW], BF16, tag="o")
                            if n % 2 == 0:
                                with tc.high_priority(-1500):
                                    nc.scalar.activation(o_t[:], po[:], AF.Copy)
                                nc.sync.dma_start(out_d[b, m, n], o_t[:])
                            else:
                                nc.vector.tensor_copy(o_t[:], po[:])
                                nc.gpsimd.dma_start(out_d[b, m, n], o_t[:])

                # ---- attention + output projection, software-pipelined:
                # phase C of the previously processed window runs between
                # attention windows so output DMA overlaps compute.  The
                # cheapest window (0) goes last to minimize the tail.
                if not late_dmas_done:
                    nc.sync.dma_start(ow_sb[:], ow)
                    late_dmas_done.append(True)
                with nc.named_scope(f"BC{b}"):
                    wins = [1, 2, 3, 0] if n_w == 4 else list(range(1, n_w)) + [0]
                    pending = []
                    for idx, w in enumerate(wins):
                        pending.append(attend_win(0, w))
                        if len(pending) > 1:
                            pending.pop(0)()
                        if idx > 0:
                            phase_c_win(wins[idx - 1], half=0)
                        pending.append(attend_win(1, w))
                        if len(pending) > 1:
                            pending.pop(0)()
                        if idx > 0:
                            phase_c_win(wins[idx - 1], half=1)
                    pending.pop(0)()
                    phase_c_win(wins[-1])

    nc.compile()
    return nc


def _host_prep(hidden_states, q_w, k_w, v_w, o_w):
    """Build the 8 per-core input maps (and shared constant tensors)."""
    T = hidden_states.shape[1]
    f32 = np.float32

    n_w = T // TQW
    KT = C // P
    # [B, T, C] -> hs^T blocked per (partition, window, k-quarter):
    # [B, P, n_w, KT//4, 4, TQW]
    hstT = hidden_states.transpose(0, 2, 1)  # [B, C, T]
    hst = np.ascontiguousarray(
        hstT.reshape(B, KT // 4, 4, P, n_w, TQW).transpose(0, 3, 4, 1, 2, 5)
    ).astype(f32, copy=False)

    def wblk(w_slice):
        # [DPC, C] row-slice -> w^T blocked [P, KT, DPC]
        return np.ascontiguousarray(
            w_slice.T.reshape(KT, P, DPC).transpose(1, 0, 2)
        ).astype(f32, copy=False)

    inv_freq = 1.0 / (ROPE_BASE ** (np.arange(0, D, 2, dtype=np.float64) / D))
    t_ar = np.arange(T, dtype=np.float64)
    freqs = t_ar[:, None] * inv_freq[None, :]  # [T, D/2]
    cos_td = np.concatenate([np.cos(freqs), np.cos(freqs)], axis=-1)  # [T, D]
    sin_td = np.concatenate([np.sin(freqs), np.sin(freqs)], axis=-1)
    cos_t = np.ascontiguousarray(cos_td.T).astype(f32)  # [D, T]
    sin_t = np.ascontiguousarray(sin_td.T).astype(f32)

    # rotate_half as a matmul: rh = R @ x ; rperm = R^T (lhsT operand).
    rperm = np.zeros((D, D), dtype=f32)
    half = D // 2
    for j in range(half):
        rperm[2 * j + 1, j] = -1.0
    for j in range(half, D):
        rperm[2 * (j - half), j] = 1.0

    ones = np.ones((P, 1), dtype=f32)

    # masks[:, 0:128]: boundary-block triangle (col >= row).
    # masks[:, 128:384]: off=3 wide mask over computed cols [256:512]:
    # valid iff tq >= 384 + p, i.e. local col j >= 128 + p.
    masks = np.zeros((P, 384), dtype=f32)
    y = np.arange(P)[:, None]
    masks[:, 0:128] = (np.arange(128)[None, :] >= y).astype(f32)
    masks[:, 128:384] = (np.arange(256)[None, :] >= 128 + y).astype(f32)

    import ml_dtypes
    bf16 = ml_dtypes.bfloat16
    hst = hst.astype(bf16)

    in_maps = []
    for c in range(N_CORES):
        rs, re = c * DPC, (c + 1) * DPC
        in_maps.append(
            {
                "hst": hst,
                "wq_t": wblk(q_w[rs:re, :]).astype(bf16),
                "wk_t": wblk(k_w[rs:re, :]).astype(bf16),
                "wv_t": wblk(v_w[rs:re, :]).astype(bf16),
                "ow_t": np.ascontiguousarray(o_w[:, rs:re].T.reshape(HPC, P, C).transpose(1, 0, 2)),
                "cos_t": cos_t,
                "sin_t": sin_t,
                "rperm": rperm,
                "ones": ones,
                "masks": masks,
            }
        )
    return in_maps


_NC_CACHE = {}


def _get_nc(T):
    if T not in _NC_CACHE:
        _NC_CACHE[T] = _build_nc(T)
    return _NC_CACHE[T]


def kernel(hidden_states, q_w, k_w, v_w, o_w, **run_kwargs):
    hidden_states = np.asarray(hidden_states, dtype=np.float32)
    q_w = np.asarray(q_w, dtype=np.float32)
    k_w = np.asarray(k_w, dtype=np.float32)
    v_w = np.asarray(v_w, dtype=np.float32)
    o_w = np.asarray(o_w, dtype=np.float32)
    T = hidden_states.shape[1]
    nc = _get_nc(T)
    in_maps = _host_prep(hidden_states, q_w, k_w, v_w, o_w)
    res = bass_utils.run_bass_kernel_spmd(
        nc, in_maps, core_ids=list(range(N_CORES)), **run_kwargs
    )
    out = np.zeros((B, T // P, C // TQW, P, TQW), dtype=np.float32)
    for r in res.results:
        out += np.asarray(r["out_p"]).astype(np.float32)
    kernel.last_results = res
    return (
        out.transpose(0, 1, 3, 2, 4).reshape(B, T, C).astype(np.float32)
    )



# revision 13
# speedup vs baseline: 1.1621x; 1.1621x over previous
"""Trainium2 Bass kernel: causal multi-head attention with RoPE.

Reference computation (B=2, T=2048, C=2048, H=16, D=128, fp32):
    q/k/v = hs @ {q,k,v}_w^T ; RoPE(q), RoPE(k)
    out   = softmax(causal(q k^T / sqrt(D))) v @ o_w^T

Sharding: tensor-parallel over heads — each of the 8 cores owns 2 heads.
Each core computes its heads' projections + attention and a partial output
projection; the host sums the 8 partials (bf16 partials, fp32 sum).

Per-core device pipeline (all matmuls in float32r = full-rate fp32):
  A) stream hs^T chunks; qT/kT in [d, t] layout (per-window tiles); v in
     [t, d] layout; RoPE (rotate_half as a +-1 permutation matmul +
     cos/sin elementwise).  The very first window/batch uses per-chunk
     tiles and DMAs spread over 4 engine queues so the first matmul
     starts ~2.5us in instead of ~20us.
  B) scores TRANSPOSED [tk, tq] with causal N-trimming: diagonal k-tiles
     only compute columns tq >= 128*off (min N=256 to keep fp32r at
     1 cyc/row); exp on ACT; masking only the boundary 128-wide block.
     Softmax denominator WITHOUT per-tile ones-matmuls: e-tiles are
     pair-summed into two accumulators (bulk tiles on DVE, the 4
     diagonal tiles on GPSIMD), then TWO chained ones-matmuls per
     window produce the den row; DVE IEEE reciprocal on the row; gpsimd
     partition-broadcast; the normalize multiply is FUSED into the
     PSUM->SBUF attnT copy.  Windows processed [1,2,3,0] so the final
     (tail-exposed) window is the cheapest one.
  C) output projection interleaved between attention windows; partial
     [t, c] tiles cast to bf16 and DMA'd out (halves the 33.5MB/core
     output traffic; host sums in fp32).
"""

import math
import sys

if "/opt/trn_rl_repo" not in sys.path:
    sys.path.insert(0, "/opt/trn_rl_repo")

import numpy as np

import concourse.bass as bass
import concourse.mybir as mybir
import concourse.tile as tile
from concourse import bacc, bass_utils

F32 = mybir.dt.float32
F32R = mybir.dt.float32r
BF16 = mybir.dt.bfloat16
AF = mybir.ActivationFunctionType
MULT = mybir.AluOpType.mult
ADD = mybir.AluOpType.add
# v2: bf16 data plane everywhere (2x DVE throughput via the 2x_1p perf
# mode, exact causal trim without the fp32r N>=256 constraint, half the
# SBUF traffic), reciprocal_approx_fast for the softmax denominator
# (~5x faster than InstReciprocal), deeper hst prefetch (bufs=8), and
# v-tile PSUM->SBUF copies on GPSIMD to relieve the ACT engine.

B = 2
C = 2048
H = 16
D = 128
N_CORES = 8
HPC = H // N_CORES  # heads per core
DPC = HPC * D  # channels per core (256)
ROPE_BASE = 10000.0
P = 128  # partitions
TQW = 512  # tq window (matmul free dim)
TCH = 256  # hs^T chunk width in t


def _build_nc(T: int = 2048):
    """Build the per-core Bass program (SPMD: same program, per-core data)."""
    KT = C // P  # 16 k-tiles over the contraction dim c
    n_w = T // TQW  # tq windows per (b, h)
    spw = TQW // P  # 128-row subtiles per window (4)
    scale = 1.0 / math.sqrt(D)

    nc = bacc.Bacc(trn_type="TRN2", target_bir_lowering=False, debug=False)

    hst = nc.dram_tensor("hst", [B, P, T // TQW, KT // 4, 4, TQW], BF16, kind="ExternalInput").ap()
    wq = nc.dram_tensor("wq_t", [P, KT, DPC], BF16, kind="ExternalInput").ap()
    wk = nc.dram_tensor("wk_t", [P, KT, DPC], BF16, kind="ExternalInput").ap()
    wv = nc.dram_tensor("wv_t", [P, KT, DPC], BF16, kind="ExternalInput").ap()
    ow = nc.dram_tensor("ow_t", [P, HPC, C], BF16, kind="ExternalInput").ap()
    cos_d = nc.dram_tensor("cos_t", [D, T], BF16, kind="ExternalInput").ap()
    sin_d = nc.dram_tensor("sin_t", [D, T], BF16, kind="ExternalInput").ap()
    rp_d = nc.dram_tensor("rperm", [D, D], BF16, kind="ExternalInput").ap()
    ones_d = nc.dram_tensor("ones", [P, 1], BF16, kind="ExternalInput").ap()
    msk_d = nc.dram_tensor("masks", [P, P], BF16, kind="ExternalInput").ap()
    out_d = nc.dram_tensor("out_p", [B, T // P, C // TQW, P, TQW], BF16, kind="ExternalOutput").ap()

    with tile.TileContext(nc) as tc:
        with (
            tc.tile_pool(name="consts", bufs=1) as consts,
            tc.tile_pool(name="hst", bufs=8) as hstp,
            tc.tile_pool(name="qkv", bufs=1) as qkvp,
            tc.tile_pool(name="exp", bufs=6) as expp,
            tc.tile_pool(name="esum", bufs=2) as esump,
            tc.tile_pool(name="bc", bufs=3) as bcp,
            tc.tile_pool(name="small", bufs=2) as smallp,
            tc.tile_pool(name="outp", bufs=8) as outp,
            tc.tile_pool(name="psA", bufs=4, space="PSUM") as psA,
            tc.tile_pool(name="psB", bufs=4, space="PSUM") as psB,
        ):
            # ---- resident constants -------------------------------------
            # First 4 contraction chunks of each weight are separate tiles
            # (fine-grained arrival for the kernel head); the remaining 12
            # keep quarter granularity.
            wq_c = [consts.tile([P, DPC], BF16, tag=f"wqc{k}", name=f"wqc{k}") for k in range(4)]
            wk_c = [consts.tile([P, DPC], BF16, tag=f"wkc{k}", name=f"wkc{k}") for k in range(4)]
            wv_c = [consts.tile([P, DPC], BF16, tag=f"wvc{k}", name=f"wvc{k}") for k in range(4)]
            wq_q = [consts.tile([P, 4, DPC], BF16, tag=f"wqq{i}", name=f"wqq{i}") for i in range(1, 4)]
            wk_q = [consts.tile([P, 4, DPC], BF16, tag=f"wkq{i}", name=f"wkq{i}") for i in range(1, 4)]
            wv_q = [consts.tile([P, 4, DPC], BF16, tag=f"wvq{i}", name=f"wvq{i}") for i in range(1, 4)]
            ow_sb = consts.tile([P, HPC, C], BF16, tag="ow")
            cos_sb = consts.tile([D, T], BF16, tag="cos")
            sin_sb = consts.tile([D, T], BF16, tag="sin")
            msk_sb = consts.tile([P, P], BF16, tag="msk")
            ones_sb = consts.tile([P, 1], BF16, tag="ones")
            rp_sb = consts.tile([D, D], BF16, tag="rp")

            def wslc(w_c, w_q, k, h=None):
                if k < 4:
                    t = w_c[k]
                    return t[:, bass.ts(h, D)] if h is not None else t[:]
                t = w_q[k // 4 - 1]
                return (
                    t[:, k % 4, bass.ts(h, D)] if h is not None else t[:, k % 4, :]
                )

            # Critical-path-first DMA order, spread over 4 issue queues so
            # the first matmuls (k=0..3 of window 0) can start early.  The
            # first window's hs^T quarters are DMA'd per chunk (4 smaller
            # transfers into slices of each quarter tile).
            pre_tiles = [
                hstp.tile([P, 4, TQW], BF16, tag="hst", name="ht_pre")
                for _ in range(4)
            ]
            nc.scalar.dma_start(wq_c[0][:], wq[:, 0, :])
            nc.gpsimd.dma_start(wk_c[0][:], wk[:, 0, :])
            nc.sync.dma_start(pre_tiles[0][:, 0, :], hst[0, :, 0, 0, 0, :])
            nc.scalar.dma_start(wq_c[1][:], wq[:, 1, :])
            nc.gpsimd.dma_start(wk_c[1][:], wk[:, 1, :])
            nc.sync.dma_start(pre_tiles[0][:, 1, :], hst[0, :, 0, 0, 1, :])
            nc.scalar.dma_start(wq_c[2][:], wq[:, 2, :])
            nc.scalar.dma_start(wk_c[2][:], wk[:, 2, :])
            nc.sync.dma_start(pre_tiles[0][:, 2, :], hst[0, :, 0, 0, 2, :])
            nc.scalar.dma_start(wq_c[3][:], wq[:, 3, :])
            nc.scalar.dma_start(wk_c[3][:], wk[:, 3, :])
            nc.sync.dma_start(pre_tiles[0][:, 3, :], hst[0, :, 0, 0, 3, :])
            nc.gpsimd.dma_start(wk_q[0][:], wk[:, bass.ts(1, 4), :])
            nc.scalar.dma_start(wq_q[0][:], wq[:, bass.ts(1, 4), :])
            for k in range(4, 8):
                nc.sync.dma_start(pre_tiles[1][:, k % 4, :], hst[0, :, 0, 1, k % 4, :])
            nc.gpsimd.dma_start(wk_q[1][:], wk[:, bass.ts(2, 4), :])
            nc.scalar.dma_start(wq_q[1][:], wq[:, bass.ts(2, 4), :])
            nc.sync.dma_start(pre_tiles[2][:], hst[0, :, 0, 2, :, :])
            nc.gpsimd.dma_start(wk_q[2][:], wk[:, bass.ts(3, 4), :])
            nc.scalar.dma_start(wq_q[2][:], wq[:, bass.ts(3, 4), :])
            nc.sync.dma_start(pre_tiles[3][:], hst[0, :, 0, 3, :, :])
            nc.scalar.dma_start(rp_sb[:], rp_d)
            for k in range(4):
                nc.sync.dma_start(wv_c[k][:], wv[:, k, :])
            for i in range(3):
                nc.sync.dma_start(wv_q[i][:], wv[:, bass.ts(i + 1, 4), :])
            nc.scalar.dma_start(cos_sb[:], cos_d)
            nc.scalar.dma_start(sin_sb[:], sin_d)
            nc.scalar.dma_start(msk_sb[:], msk_d)
            nc.scalar.dma_start(ones_sb[:], ones_d)
            late_dmas_done = []

            for b in range(B):
                # Per-window q/k tiles: fine-grained deps (a window's
                # consumers only wait on that window's producers).
                q_t = [
                    [qkvp.tile([P, TQW], BF16, tag=f"q{h}w{w}", name=f"q{h}w{w}") for w in range(n_w)]
                    for h in range(HPC)
                ]
                k_t = [
                    [qkvp.tile([P, TQW], BF16, tag=f"k{h}w{w}", name=f"k{h}w{w}") for w in range(n_w)]
                    for h in range(HPC)
                ]
                v_sb = qkvp.tile([P, T // P, DPC], BF16, tag="v")

                # ---- phase A: projections + RoPE ------------------------
                def rope(w, b=b):
                    sl = bass.ts(w, TQW)
                    for h in range(HPC):
                        for x_t in (q_t, k_t):
                            x = x_t[h][w]
                            rh = psB.tile([P, TQW], F32, tag="psB", name="rh")
                            nc.tensor.matmul(
                                rh[:], rp_sb[:], x[:], start=True, stop=True
                            )
                            # t1 = x*cos (all-bf16: 2x DVE); rh2 = rh*sin
                            # (psum f32 x bf16 -> bf16); x = t1+rh2 (2x DVE)
                            t1 = smallp.tile([P, TQW], BF16, tag="t1")
                            nc.vector.tensor_tensor(
                                t1[:], x[:], cos_sb[:, sl], op=MULT
                            )
                            rh2 = smallp.tile([P, TQW], BF16, tag="t2")
                            nc.vector.tensor_tensor(rh2[:], rh[:], sin_sb[:, sl], op=MULT)
                            nc.vector.tensor_tensor(x[:], t1[:], rh2[:], op=ADD)

                ctx_a = nc.named_scope(f"A{b}"); ctx_a.__enter__()
                for w in range(n_w):
                    if b == 0 and w == 0:
                        hts = pre_tiles
                    else:
                        hts = []
                        for qi in range(4):
                            ht = hstp.tile([P, 4, TQW], BF16, tag="hst", name="ht")
                            nc.sync.dma_start(ht[:], hst[b, :, w, qi, :, :])
                            hts.append(ht)
                    hsl = [hts[k // 4][:, k % 4, :] for k in range(KT)]
                    hsl_sub = lambda k, sub, hts=hts: hts[k // 4][:, k % 4, bass.ts(sub, P)]
                    pq = [psA.tile([P, TQW], F32, tag="psA", name="pq") for _ in range(HPC)]
                    pk = [psA.tile([P, TQW], F32, tag="psA", name="pk") for _ in range(HPC)]
                    for k in range(KT):
                        for h in range(HPC):
                            for pt, w_cq in ((pq[h], (wq_c, wq_q)), (pk[h], (wk_c, wk_q))):
                                nc.tensor.matmul(
                                    pt[:],
                                    wslc(w_cq[0], w_cq[1], k, h),
                                    hsl[k],
                                    start=(k == 0),
                                    stop=(k == KT - 1),
                                )
                    # Rank the psum->sbuf copies later so attention's first
                    # exps win the ACT queue at the phase A->B transition
                    # (deps still force early-window copies on time).
                    with tc.high_priority(-2000):
                        for h in range(HPC):
                            nc.scalar.activation(q_t[h][w][:], pq[h][:], AF.Copy)
                            nc.scalar.activation(k_t[h][w][:], pk[h][:], AF.Copy)
                    pv4 = [
                        psB.tile([P, DPC], F32, tag="psB", name="pv4")
                        for _ in range(spw)
                    ]
                    for k in range(KT):
                        for sub in range(spw):
                            nc.tensor.matmul(
                                pv4[sub][:],
                                hsl_sub(k, sub),
                                wslc(wv_c, wv_q, k),
                                start=(k == 0),
                                stop=(k == KT - 1),
                            )
                    with tc.high_priority(-2000):
                        for sub in range(spw):
                            # GPSIMD cannot read PSUM; DVE takes these
                            # (it is well under budget after the bf16 move)
                            nc.vector.tensor_copy(
                                v_sb[:, w * spw + sub, :], pv4[sub][:]
                            )
                    rope(w)
                ctx_a.__exit__(None, None, None)

                # ---- phase B: attention -------------------------------
                # Diagonal k-tile column trim: tile with offset `off`
                # (0..3) only needs columns tq >= 128*off (bf16 matmuls
                # have no minimum-N penalty, so the trim is exact).  The
                # boundary 128-wide block at [c0:c0+128] gets the
                # triangle mask; columns above it are fully valid.
                def attend_win(h, w):
                    ntk = (w + 1) * spw

                    def qk_exp(i, h=h, w=w):
                        off = i - w * spw
                        c0 = 0 if off <= 0 else P * off
                        st = psB.tile([P, TQW], F32, tag="psB")
                        nc.tensor.matmul(
                            st[:, c0:],
                            k_t[h][i // spw][:, bass.ts(i % spw, P)],
                            q_t[h][w][:, c0:],
                            start=True,
                            stop=True,
                        )
                        e = expp.tile([P, TQW], BF16, tag="exp")
                        nc.scalar.activation(e[:, c0:], st[:, c0:], AF.Exp, scale=scale)
                        if off >= 0:
                            o0 = P * off
                            nc.vector.tensor_tensor(
                                e[:, o0:o0 + P], e[:, o0:o0 + P],
                                msk_sb[:], op=MULT,
                            )
                        return e, c0

                    fifo = [qk_exp(j) for j in range(min(3, ntk))]
                    pv = psA.tile([P, TQW], F32, tag="psA")
                    den = psA.tile([P, TQW], F32, tag="psA")
                    nfull = ntk - spw  # leading full-width tiles
                    # Denominator: DVE pair-sums of full-width e-tiles feed
                    # one ones-matmul per pair; the 4 diagonal tiles get
                    # individual N-trimmed ones-matmuls.  Jobs are emitted
                    # one behind the PV stream so nothing stalls.  Natural
                    # order puts a full-width operand first (pair0, or
                    # w=0's off-0 tile), as required for start=True.
                    pair_pend = None
                    jobs = []  # (ap, c0) pending den matmuls
                    n_jobs_total = nfull // 2 + spw
                    emitted = [0]

                    def den_mm():
                        ap, c0 = jobs.pop(0)
                        nc.tensor.matmul(
                            den[:1, c0:],
                            ones_sb[:],
                            ap,
                            start=(emitted[0] == 0),
                            stop=(emitted[0] == n_jobs_total - 1),
                        )
                        emitted[0] += 1

                    for i in range(ntk):
                        if i + 3 < ntk:
                            fifo.append(qk_exp(i + 3))
                        e, c0 = fifo.pop(0)
                        nc.tensor.matmul(
                            pv[:, c0:],
                            v_sb[:, i, bass.ts(h, D)],
                            e[:, c0:],
                            start=(i == 0),
                            stop=(i == ntk - 1),
                        )
                        if i < nfull:
                            if pair_pend is None:
                                pair_pend = e
                            else:
                                pr = esump.tile(
                                    [P, TQW], BF16, tag="pair", name="pair"
                                )
                                nc.vector.tensor_tensor(
                                    pr[:],
                                    pair_pend[:],
                                    e[:],
                                    op=ADD,
                                )
                                pair_pend = None
                                jobs.append((pr[:], 0))
                        else:
                            jobs.append((e[:, c0:], c0))
                        # drain den jobs, keeping two in flight for slack
                        while len(jobs) > 2:
                            den_mm()
                    while jobs:
                        den_mm()

                    # reciprocal immediately (frees the den PSUM slot);
                    # broadcast + normalize are deferred one attend-step by
                    # the caller to avoid engine convoys.  approx_fast is
                    # ~5x faster than InstReciprocal and accurate to ~18
                    # bits -- far beyond what softmax normalization needs.
                    bc = bcp.tile([P, TQW], F32, tag="bc", name="bc")
                    nc.vector.reciprocal_approx_fast(bc[:1, :], den[:1, :])

                    def finalize(h=h, w=w, pv=pv, bc=bc):
                        nc.gpsimd.partition_broadcast(bc[:], bc[:1, :])
                        nc.vector.tensor_tensor(
                            q_t[h][w][:], pv[:], bc[:], op=MULT
                        )
                    return finalize

                def phase_c_win(w, half=None):
                    ms = range(w * spw, (w + 1) * spw)
                    if half is not None:
                        ms = ms[: len(ms) // 2] if half == 0 else ms[len(ms) // 2 :]
                    for m in ms:
                        for n in range(C // TQW):
                            pool = psA if n % 2 == 0 else psB
                            po = pool.tile([P, TQW], F32, tag=pool.name, name="po")
                            for h in range(HPC):
                                nc.tensor.matmul(
                                    po[:],
                                    q_t[h][m // spw][:, bass.ts(m % spw, P)],
                                    ow_sb[:, h, bass.ts(n, TQW)],
                                    start=(h == 0),
                                    stop=(h == HPC - 1),
                                )
                            o_t = outp.tile([P, TQW], BF16, tag="o")
                            if n % 2 == 0:
                                with tc.high_priority(-1500):
                                    nc.scalar.activation(o_t[:], po[:], AF.Copy)
                                nc.sync.dma_start(out_d[b, m, n], o_t[:])
                            else:
                                nc.vector.tensor_copy(o_t[:], po[:])
                                nc.gpsimd.dma_start(out_d[b, m, n], o_t[:])

                # ---- attention + output projection, software-pipelined:
                # phase C of the previously processed window runs between
                # attention windows so output DMA overlaps compute.  The
                # cheapest window (0) goes last to minimize the tail.
                if not late_dmas_done:
                    nc.sync.dma_start(ow_sb[:], ow)
                    late_dmas_done.append(True)
                with nc.named_scope(f"BC{b}"):
                    wins = [1, 2, 3, 0] if n_w == 4 else list(range(1, n_w)) + [0]
                    pending = []
                    for idx, w in enumerate(wins):
                        pending.append(attend_win(0, w))
                        if len(pending) > 1:
                            pending.pop(0)()
                        if idx > 0:
                            phase_c_win(wins[idx - 1], half=0)
                        pending.append(attend_win(1, w))
                        if len(pending) > 1:
                            pending.pop(0)()
                        if idx > 0:
                            phase_c_win(wins[idx - 1], half=1)
                    pending.pop(0)()
                    phase_c_win(wins[-1])

    nc.compile()
    return nc


def _host_prep(hidden_states, q_w, k_w, v_w, o_w):
    """Build the 8 per-core input maps (and shared constant tensors)."""
    T = hidden_states.shape[1]
    f32 = np.float32

    n_w = T // TQW
    KT = C // P
    # [B, T, C] -> hs^T blocked per (partition, window, k-quarter):
    # [B, P, n_w, KT//4, 4, TQW]
    hstT = hidden_states.transpose(0, 2, 1)  # [B, C, T]
    hst = np.ascontiguousarray(
        hstT.reshape(B, KT // 4, 4, P, n_w, TQW).transpose(0, 3, 4, 1, 2, 5)
    ).astype(f32, copy=False)

    def wblk(w_slice):
        # [DPC, C] row-slice -> w^T blocked [P, KT, DPC]
        return np.ascontiguousarray(
            w_slice.T.reshape(KT, P, DPC).transpose(1, 0, 2)
        ).astype(f32, copy=False)

    inv_freq = 1.0 / (ROPE_BASE ** (np.arange(0, D, 2, dtype=np.float64) / D))
    t_ar = np.arange(T, dtype=np.float64)
    freqs = t_ar[:, None] * inv_freq[None, :]  # [T, D/2]
    cos_td = np.concatenate([np.cos(freqs), np.cos(freqs)], axis=-1)  # [T, D]
    sin_td = np.concatenate([np.sin(freqs), np.sin(freqs)], axis=-1)
    cos_t = np.ascontiguousarray(cos_td.T).astype(f32)  # [D, T]
    sin_t = np.ascontiguousarray(sin_td.T).astype(f32)

    # rotate_half as a matmul: rh = R @ x ; rperm = R^T (lhsT operand).
    rperm = np.zeros((D, D), dtype=f32)
    half = D // 2
    for j in range(half):
        rperm[2 * j + 1, j] = -1.0
    for j in range(half, D):
        rperm[2 * (j - half), j] = 1.0

    ones = np.ones((P, 1), dtype=f32)

    # masks: boundary-block triangle (col >= row).
    masks = (np.arange(P)[None, :] >= np.arange(P)[:, None]).astype(f32)

    import ml_dtypes
    bf16 = ml_dtypes.bfloat16
    hst = hst.astype(bf16)

    in_maps = []
    for c in range(N_CORES):
        rs, re = c * DPC, (c + 1) * DPC
        in_maps.append(
            {
                "hst": hst,
                "wq_t": wblk(q_w[rs:re, :]).astype(bf16),
                "wk_t": wblk(k_w[rs:re, :]).astype(bf16),
                "wv_t": wblk(v_w[rs:re, :]).astype(bf16),
                "ow_t": np.ascontiguousarray(
                    o_w[:, rs:re].T.reshape(HPC, P, C).transpose(1, 0, 2)
                ).astype(bf16),
                "cos_t": cos_t.astype(bf16),
                "sin_t": sin_t.astype(bf16),
                "rperm": rperm.astype(bf16),
                "ones": ones.astype(bf16),
                "masks": masks.astype(bf16),
            }
        )
    return in_maps


_NC_CACHE = {}


def _get_nc(T):
    if T not in _NC_CACHE:
        _NC_CACHE[T] = _build_nc(T)
    return _NC_CACHE[T]


def kernel(hidden_states, q_w, k_w, v_w, o_w, **run_kwargs):
    hidden_states = np.asarray(hidden_states, dtype=np.float32)
    q_w = np.asarray(q_w, dtype=np.float32)
    k_w = np.asarray(k_w, dtype=np.float32)
    v_w = np.asarray(v_w, dtype=np.float32)
    o_w = np.asarray(o_w, dtype=np.float32)
    T = hidden_states.shape[1]
    nc = _get_nc(T)
    in_maps = _host_prep(hidden_states, q_w, k_w, v_w, o_w)
    res = bass_utils.run_bass_kernel_spmd(
        nc, in_maps, core_ids=list(range(N_CORES)), **run_kwargs
    )
    out = np.zeros((B, T // P, C // TQW, P, TQW), dtype=np.float32)
    for r in res.results:
        out += np.asarray(r["out_p"]).astype(np.float32)
    kernel.last_results = res
    return (
        out.transpose(0, 1, 3, 2, 4).reshape(B, T, C).astype(np.float32)
    )

